# revision 12
# baseline (speedup 1.0000x reference)
"""Causal GQA self-attention (B=2, S=2048, D=2048, 32 Q heads / 8 KV heads,
hd=64, RoPE) on 8 TRN2 NeuronCores.

Sharding: 2-way data parallel over batch x 4-way tensor parallel over heads.
Core c handles batch b=c//4 and head group g=c%4 (8 Q heads, 2 KV heads).
Each core computes a partial out-projection from its own heads (row shard of
Wo); partials are ReduceScattered (f32 add) within each 4-core group so each
core ends with a 512-row seq slice of the final output. Host reassembles.

Heads are paired (t, t+4) on the 128 SBUF partitions so that a Q head's
partition range matches its KV head's range in kT (heads 0-3 -> kv0 on
partitions 0:64, heads 4-7 -> kv1 on 64:128), avoiding partition-swap copies.

Phases are software-pipelined over 512-seq blocks: step sb emits QKV+RoPE for
block sb interleaved with attention for block sb-1 and the out-projection +
ReduceScatter for block sb-2, keeping PE busy while the Activation engine
works through softmax exps.

Matmuls run bf16 x bf16 -> fp32 PSUM; softmax/normalization in fp32.
"""
import sys
sys.path.insert(0, "/opt/trn_rl_repo")
import numpy as np
import ml_dtypes
import concourse.bass as bass
import concourse.mybir as mybir
import concourse.tile as tile
from concourse import bacc
from concourse.bass_utils import run_bass_kernel_spmd
from concourse.masks import make_identity

MODEL_DIM = 2048
SEQ = 2048
HEAD_DIM = 64
ROPE_BASE = 10000.0
BATCH = 2
NCORES = 8
GROUPS = [[0, 1, 2, 3], [4, 5, 6, 7]]
QF = 512   # q features per core (8 heads * 64)
KF = 128   # kv features per core (2 kv heads * 64)

f32 = mybir.dt.float32
bf16 = mybir.dt.bfloat16
ACTF = mybir.ActivationFunctionType
BF = ml_dtypes.bfloat16

_cache = {}


def _build_kernel():
    nc = bacc.Bacc(None, target_bir_lowering=False, debug=False,
                   num_devices=NCORES)
    xT = nc.dram_tensor("xT", [MODEL_DIM, SEQ], bf16, kind="ExternalInput").ap()
    wq = nc.dram_tensor("wq", [MODEL_DIM, QF], bf16, kind="ExternalInput").ap()
    wk = nc.dram_tensor("wk", [MODEL_DIM, KF], bf16, kind="ExternalInput").ap()
    wv = nc.dram_tensor("wv", [MODEL_DIM, KF], bf16, kind="ExternalInput").ap()
    wo = nc.dram_tensor("wo", [QF, MODEL_DIM], bf16, kind="ExternalInput").ap()
    p2 = nc.dram_tensor("p2", [128, 128], bf16, kind="ExternalInput").ap()
    cosr = nc.dram_tensor("cosr", [128, SEQ], bf16, kind="ExternalInput").ap()
    sinr = nc.dram_tensor("sinr", [128, SEQ], bf16, kind="ExternalInput").ap()
    masks = nc.dram_tensor("masks", [128, 4 * 512], bf16, kind="ExternalInput").ap()
    out = nc.dram_tensor("out", [512, MODEL_DIM], bf16, kind="ExternalOutput").ap()

    with tile.TileContext(nc) as tc:
        from contextlib import ExitStack
        with ExitStack() as ctx:
            persist = ctx.enter_context(tc.tile_pool(name="persist", bufs=1))
            consts = ctx.enter_context(tc.tile_pool(name="consts", bufs=1))
            dram = ctx.enter_context(tc.tile_pool(name="dram", bufs=1, space="DRAM"))
            xts = ctx.enter_context(tc.tile_pool(name="xts", bufs=2))
            ropep = ctx.enter_context(tc.tile_pool(name="ropep", bufs=3))
            etp = ctx.enter_context(tc.tile_pool(name="etp", bufs=4))
            rcpp = ctx.enter_context(tc.tile_pool(name="rcpp", bufs=3))
            p3sb = ctx.enter_context(tc.tile_pool(name="p3sb", bufs=4))
            accp = ctx.enter_context(tc.tile_pool(name="accp", bufs=2, space="PSUM"))
            psp = ctx.enter_context(tc.tile_pool(name="psp", bufs=2, space="PSUM"))
            pavp = ctx.enter_context(tc.tile_pool(name="pavp", bufs=2, space="PSUM"))

            # persistent activations
            qT = [persist.tile([128, SEQ], bf16, tag=f"qT{t}", name=f"qT{t}")
                  for t in range(4)]
            kT = persist.tile([128, SEQ], bf16, tag="kT")
            # V natural [seq-chunk, 132]: kv0 cols 0:64, ones 64:65,
            # kv1 cols 66:130, ones 130:131
            V1 = persist.tile([128, 16, 132], bf16, tag="V1")
            # feature-major attention output y^T, pair-tile row order
            y_sb = persist.tile([128, 4, SEQ], bf16, tag="y_sb")

            # constants / weights
            p2_sb = consts.tile([128, 128], bf16, tag="p2")
            ones_sb = consts.tile([1, 64], bf16, tag="ones")
            ident = consts.tile([128, 128], bf16, tag="ident")
            cos_sb = consts.tile([128, SEQ], bf16, tag="cos")
            sin_sb = consts.tile([128, SEQ], bf16, tag="sin")
            masks_sb = consts.tile([128, 4 * 512], bf16, tag="masks")
            wq_sb = consts.tile([128, 16, QF], bf16, tag="wq")
            wk_sb = consts.tile([128, 16, KF], bf16, tag="wk")
            wv_sb = consts.tile([128, 16, KF], bf16, tag="wv")
            wo_sb = consts.tile([128, 4, MODEL_DIM], bf16, tag="wo")

            nc.sync.dma_start(p2_sb[:], p2[:])
            make_identity(nc, ident[:])
            nc.sync.dma_start(cos_sb[:], cosr[:])
            nc.sync.dma_start(sin_sb[:], sinr[:])
            nc.sync.dma_start(masks_sb[:], masks[:])
            nc.sync.dma_start(wq_sb[:], wq.rearrange("(c p) m -> p c m", p=128))
            nc.sync.dma_start(wk_sb[:], wk.rearrange("(c p) m -> p c m", p=128))
            nc.sync.dma_start(wv_sb[:], wv.rearrange("(c p) m -> p c m", p=128))
            nc.sync.dma_start(wo_sb[:], wo.rearrange("(c p) n -> p c n", p=128))
            nc.vector.memset(ones_sb[:], 1.0)
            nc.vector.memset(V1[:, :, 64:65], 1.0)
            nc.vector.memset(V1[:, :, 130:131], 1.0)

            partial = dram.tile([SEQ, MODEL_DIM], bf16)
            rs_out = dram.tile([512, MODEL_DIM], bf16)

            xt_tiles = {}

            def load_xt(sb):
                xt = xts.tile([128, 16, 512], bf16, tag="xt")
                for dc in range(16):
                    nc.sync.dma_start(
                        xt[:, dc, :],
                        xT[dc * 128:(dc + 1) * 128, sb * 512:(sb + 1) * 512])
                xt_tiles[sb] = xt

            load_xt(0)

            def p1_units(sb):
                """QKV projections + RoPE for seq block sb. Yields ~12 units."""
                ssl = slice(sb * 512, (sb + 1) * 512)
                xt = xt_tiles[sb]
                if sb + 1 < 4:
                    load_xt(sb + 1)

                def rope_copy(acc):
                    # frees the accumulation PSUM promptly (Act engine)
                    tq = ropep.tile([128, 512], bf16, tag="tq")
                    nc.scalar.activation(tq[:], acc[:], ACTF.Copy)
                    return tq

                def rope_rest(tq, dst):
                    # rope(z) = z*cos + (z@P)*sin; sin table is half-duplicated
                    # so pre-multiplying by sin before the rotation is exact
                    a = ropep.tile([128, 512], bf16, tag="a")
                    b = ropep.tile([128, 512], bf16, tag="b")
                    with nc.allow_low_precision(reason="bf16 rope"):
                        nc.vector.tensor_mul(a[:], tq[:], cos_sb[:, ssl])
                        nc.vector.tensor_mul(b[:], tq[:], sin_sb[:, ssl])
                    rot = accp.tile([128, 512], f32, tag="acc", name="rot")
                    nc.tensor.matmul(rot[:], p2_sb[:], b[:], start=True, stop=True)
                    with nc.allow_low_precision(reason="bf16 rope"):
                        nc.vector.tensor_add(dst[:, ssl], a[:], rot[:])

                # K first so attention of block sb can start earliest
                acc_k = accp.tile([128, 512], f32, tag="acc", name="acc_k")
                for dc in range(16):
                    nc.tensor.matmul(acc_k[:], wk_sb[:, dc, :], xt[:, dc, :],
                                     start=(dc == 0), stop=(dc == 15))
                prev = (rope_copy(acc_k), kT)
                yield "p1-k"
                for t in range(4):
                    acc_q = accp.tile([128, 512], f32, tag="acc", name=f"acc_q{t}")
                    for dc in range(16):
                        nc.tensor.matmul(
                            acc_q[:], wq_sb[:, dc, t * 128:(t + 1) * 128],
                            xt[:, dc, :], start=(dc == 0), stop=(dc == 15))
                    rope_rest(*prev)
                    prev = (rope_copy(acc_q), qT[t])
                    yield f"p1-q{t}"
                # V pass: natural layout [seq, kvf]
                acc_v = accp.tile([128, 512], f32, tag="acc", name="acc_v")
                for sc in range(4):
                    for dc in range(16):
                        nc.tensor.matmul(
                            acc_v[:, sc * 128:(sc + 1) * 128],
                            xt[:, dc, sc * 128:(sc + 1) * 128],
                            wv_sb[:, dc, :], start=(dc == 0), stop=(dc == 15))
                rope_rest(*prev)
                yield "p1-v"
                for sc in range(4):
                    kc = sb * 4 + sc
                    with nc.allow_low_precision(reason="bf16 v"):
                        nc.vector.tensor_copy(V1[:, kc, 0:64],
                                              acc_v[:, sc * 128:sc * 128 + 64])
                        nc.vector.tensor_copy(V1[:, kc, 66:130],
                                              acc_v[:, sc * 128 + 64:(sc + 1) * 128])
                yield "p1-vcopy"

            def p2_units(j):
                """Attention for query block j (uses K/V blocks 0..j).
                Yields 4 pair-units (2 heads each)."""
                jsl = slice(j * 512, (j + 1) * 512)
                npair = 2 * (j + 1)
                nkc = 4 * (j + 1)
                for pt in range(4):
                    for h in (pt, pt + 4):
                        kp = slice(64 * (h // 4), 64 * (h // 4) + 64)
                        qsl = qT[pt][kp, jsl]
                        # attention output accumulator [65, 512]: rows 0:64 =
                        # y^T (hd-major), row 64 = softmax denominator
                        pq = pavp.tile([65, 512], f32, tag="pav")
                        ets = []

                        def av_pair(p):
                            et = ets[p]
                            for b2 in range(2):
                                kc = 2 * p + b2
                                vsl = (V1[:, kc, 0:65] if h < 4
                                       else V1[:, kc, 66:131])
                                nc.tensor.matmul(
                                    pq[:], vsl, et[:, b2, :],
                                    start=(kc == 0), stop=(kc == nkc - 1))

                        for p in range(npair):
                            ps = psp.tile([128, 2, 512], f32, tag="ps")
                            for b2 in range(2):
                                nc.tensor.matmul(
                                    ps[:, b2, :],
                                    kT[kp, (2 * p + b2) * 128:(2 * p + b2 + 1) * 128],
                                    qsl, start=True, stop=True)
                            et = etp.tile([128, 2, 512], bf16, tag="et")
                            nc.scalar.activation(et[:], ps[:], ACTF.Exp,
                                                 scale=0.125)
                            tp = p - 2 * j
                            if tp >= 0:
                                with nc.allow_low_precision(reason="mask"):
                                    nc.vector.tensor_mul(
                                        et[:], et[:],
                                        masks_sb[:, tp * 1024:(tp + 1) * 1024])
                            ets.append(et)
                            if p >= 1:
                                av_pair(p - 1)
                        av_pair(npair - 1)

                        # normalize by fused denominator (row 64): broadcast
                        # 1/denom across 64 partitions via PE, then scale
                        rcp = rcpp.tile([1, 512], bf16, tag="rcp")
                        with nc.allow_low_precision(reason="softmax denom"):
                            nc.vector.reciprocal(rcp[:], pq[64:65, :])
                        bct = psp.tile([128, 2, 512], f32, tag="ps", name="bct")
                        nc.tensor.matmul(bct[0:64, 0, :], ones_sb[:], rcp[:],
                                         start=True, stop=True)
                        bc = rcpp.tile([64, 512], f32, tag="bc")
                        nc.vector.tensor_copy(bc[:], bct[0:64, 0, :])
                        yrow = slice(64 * (h // 4), 64 * (h // 4) + 64)
                        with nc.allow_low_precision(reason="bf16 y"):
                            nc.vector.tensor_mul(y_sb[yrow, pt, jsl],
                                                 pq[0:64, :], bc[:])
                    yield f"p2-j{j}-pair{pt}"

            def p3_units(j):
                """Partial out-projection for seq block j + ReduceScatter."""
                for sc in range(4):
                    for ob in range(4):
                        pso = accp.tile([128, 512], f32, tag="acc", name="pso")
                        for fc in range(4):
                            nc.tensor.matmul(
                                pso[:],
                                y_sb[:, fc, j * 512 + sc * 128:j * 512 + (sc + 1) * 128],
                                wo_sb[:, fc, ob * 512:(ob + 1) * 512],
                                start=(fc == 0), stop=(fc == 3))
                        pt_sb = p3sb.tile([128, 512], bf16, tag="p3c")
                        with nc.allow_low_precision(reason="bf16 partial"):
                            if (sc + ob) % 2 == 0:
                                nc.scalar.activation(pt_sb[:], pso[:], ACTF.Copy)
                            else:
                                nc.vector.tensor_copy(pt_sb[:], pso[:])
                        r0 = j * 512 + sc * 128
                        nc.sync.dma_start(
                            partial[r0:r0 + 128, ob * 512:(ob + 1) * 512],
                            pt_sb[:])
                    yield f"p3-j{j}-sc{sc}"
                nc.gpsimd.collective_compute(
                    "ReduceScatter", mybir.AluOpType.add,
                    ins=[partial[j * 512:(j + 1) * 512, :].opt()],
                    outs=[rs_out[j * 128:(j + 1) * 128, :].opt()],
                    replica_groups=GROUPS)
                nc.sync.dma_start(out[j * 128:(j + 1) * 128, :],
                                  rs_out[j * 128:(j + 1) * 128, :])
                yield f"p3-j{j}-rs"

            for step in range(6):
                gens = []
                if step < 4:
                    gens.append(p1_units(step))
                if 1 <= step <= 4:
                    gens.append(p2_units(step - 1))
                if step >= 2:
                    gens.append(p3_units(step - 2))
                while gens:
                    alive = []
                    for gunit in gens:
                        try:
                            next(gunit)
                            alive.append(gunit)
                        except StopIteration:
                            pass
                    gens = alive

    nc.compile()
    return nc


def _host_constants():
    inv_freq = (1.0 / (ROPE_BASE ** (np.arange(0, HEAD_DIM, 2, dtype=np.float32)
                                     / HEAD_DIM))).astype(np.float32)
    t = np.arange(SEQ, dtype=np.float32)
    freqs = np.outer(t, inv_freq)                      # [S, 32]
    emb = np.concatenate([freqs, freqs], axis=-1)      # [S, 64]
    cosT = np.cos(emb).astype(np.float32).T            # [64, S]
    sinT = np.sin(emb).astype(np.float32).T
    cosr = np.ascontiguousarray(np.vstack([cosT, cosT])).astype(BF)  # [128, S]
    sinr = np.ascontiguousarray(np.vstack([sinT, sinT])).astype(BF)

    # rotation matrix: rot(z)[m] = -z[m+32] (m<32), z[m-32] (m>=32); 2 blocks
    R = np.zeros((64, 64), dtype=np.float32)
    for d in range(32):
        R[d + 32, d] = -1.0
        R[d, d + 32] = 1.0
    p2 = np.zeros((128, 128), dtype=np.float32)
    p2[0:64, 0:64] = R
    p2[64:128, 64:128] = R

    k_idx = np.arange(128)[:, None]
    q_idx = np.arange(512)[None, :]
    m = np.concatenate(
        [(128 * t_ + k_idx <= q_idx).astype(np.float32) for t_ in range(4)],
        axis=1)                                        # [128, 2048]
    return cosr, sinr, p2.astype(BF), np.ascontiguousarray(m).astype(BF)


def _in_maps(x, Wq, Wk, Wv, Wo):
    cosr, sinr, p2, masks = _host_constants()
    xb = [np.ascontiguousarray(x[b].T).astype(BF) for b in range(BATCH)]
    wqb = Wq.astype(BF)
    wkb = Wk.astype(BF)
    wvb = Wv.astype(BF)
    wob = Wo.astype(BF)
    maps = []
    for c in range(NCORES):
        b, g = c // 4, c % 4
        # head-pair permutation: pair tile t holds local heads t and t+4
        wq_l = np.empty((MODEL_DIM, QF), dtype=BF)
        wo_l = np.empty((QF, MODEL_DIM), dtype=BF)
        for t in range(4):
            h0 = (g * 8 + t) * 64
            h1 = (g * 8 + t + 4) * 64
            wq_l[:, t * 128:t * 128 + 64] = wqb[:, h0:h0 + 64]
            wq_l[:, t * 128 + 64:(t + 1) * 128] = wqb[:, h1:h1 + 64]
            wo_l[t * 128:t * 128 + 64, :] = wob[h0:h0 + 64, :]
            wo_l[t * 128 + 64:(t + 1) * 128, :] = wob[h1:h1 + 64, :]
        maps.append({
            "xT": xb[b],
            "wq": np.ascontiguousarray(wq_l),
            "wk": np.ascontiguousarray(wkb[:, g * KF:(g + 1) * KF]),
            "wv": np.ascontiguousarray(wvb[:, g * KF:(g + 1) * KF]),
            "wo": np.ascontiguousarray(wo_l),
            "p2": p2, "cosr": cosr, "sinr": sinr, "masks": masks,
        })
    return maps


def kernel(x, Wq, Wk, Wv, Wo):
    x = np.asarray(x, dtype=np.float32)
    Wq = np.asarray(Wq, dtype=np.float32)
    Wk = np.asarray(Wk, dtype=np.float32)
    Wv = np.asarray(Wv, dtype=np.float32)
    Wo = np.asarray(Wo, dtype=np.float32)

    if "nc" not in _cache:
        _cache["nc"] = _build_kernel()
    nc = _cache["nc"]

    res = run_bass_kernel_spmd(nc, _in_maps(x, Wq, Wk, Wv, Wo),
                               list(range(NCORES)))
    out = np.empty((BATCH, SEQ, MODEL_DIM), dtype=np.float32)
    for c in range(NCORES):
        b, i = c // 4, c % 4
        r = np.asarray(res.results[c]["out"]).astype(np.float32)
        for j in range(4):
            out[b, j * 512 + i * 128:j * 512 + (i + 1) * 128, :] = \
                r[j * 128:(j + 1) * 128, :]
    return out
